# revision 7
# baseline (speedup 1.0000x reference)
"""Multi-head attention with random-synthesizer blend + mask, on 8 Trainium2
NeuronCores.  v5: v4 + fp8 DoubleRow q/k projections (c1 folded into the exp
scale, weights prescaled x16 to stay out of e4m3 subnormals), projection
weight DMAs staggered one chunk ahead of their matmuls, startup input DMAs
spread across the hardware DGE queues.

Sharding: data-parallel over batch (B=8 -> one batch element per core).

Per-core layouts ([partition, free]):
  - xq8/xk8: [P, 2, S] fp8 di-pair tiles (DoubleRow moving operands);
    xv: [D, S] fp16 chunks.  emsk[h,kc] = exp((1-alpha)*syn[h].T)*mask.T fp16.
  - qT/kT: [d_out, s] fp16 scaled by SCL.  v_sb: [s, H*65] fp16 with a ones
    column per head, so PV matmul row 64 yields the softmax sum.
  - Attention per (h,kc): scores_T (fp16) -> exp(scale*psum) (ACT) -> one
    emsk multiply (DVE) -> PV accumulate into pav[0:65].  PV for chunk kc-1
    is emitted after the projection fills of chunk kc (PE never waits).
  - Normalization: reciprocal_approx_fast on [33,1024] sums, DVE cast,
    rank-1 ones-matmul broadcast, one DVE multiply.
  - o-proj transposed (Wo chunks stationary, otn moving), boeff as ACT
    bias; host transposes back.  Output fp16 [D, S].
"""

import math
import sys

sys.path.insert(0, "/opt/trn_rl_repo")

import numpy as np

import concourse.tile as tile
import concourse.mybir as mybir
from concourse import bacc
from concourse.bass_utils import run_bass_kernel_spmd

B, S, D, H = 8, 1024, 1024, 16
HD = D // H  # 64
N_CORES = 8
P = 128
SC = S // P  # 8
DC = D // P  # 8
NQ = 512
VW = HD + 1  # 65: v block width incl ones column

f32 = mybir.dt.float32
fp16 = mybir.dt.float16
fp8e4 = mybir.dt.float8e4
AF = mybir.ActivationFunctionType
OP = mybir.AluOpType
DR = mybir.MatmulPerfMode.DoubleRow

SCL = 16.0  # fp8 weight prescale (keeps W out of the e4m3 subnormal range)
NJ = DC // 2  # 4: DoubleRow di-pair count

TRACE = False
TRACE_TMPDIR = None
LAST_RESULTS = None

_CACHE = {}


def _emit(nc, tc, dram, exp_scale):
    w_d = {"q": dram["wq"], "k": dram["wk"], "v": dram["wv"], "o": dram["wo"]}
    out_d = dram["out"]

    with (
        tc.tile_pool(name="pers", bufs=1) as pers,
        tc.tile_pool(name="psmm", bufs=1, space="PSUM") as psmm,
        tc.tile_pool(name="psav", bufs=1, space="PSUM") as psav,
    ):
        # ---- constants ---------------------------------------------------
        ones_h = pers.tile([33, P], fp16, tag="ones_h")
        nc.vector.memset(ones_h[:], 1.0)
        bqk_sb = {}
        for nm in ("q", "k"):
            t = pers.tile([P, DC], f32, tag=f"b{nm}", name=f"b{nm}")
            nc.gpsimd.dma_start(out=t[:], in_=dram["b" + nm].rearrange("(c p) -> p c", p=P))
            bqk_sb[nm] = t
        bo_sb = pers.tile([P, DC], f32, tag="bo_sb")
        nc.gpsimd.dma_start(out=bo_sb[:], in_=dram["boeff"].rearrange("(c p) -> p c", p=P))

        # ---- persistent activations --------------------------------------
        qT = [pers.tile([P, S], fp16, tag=f"qT{i}", name=f"qT{i}") for i in range(DC)]
        kT = [pers.tile([P, S], fp16, tag=f"kT{i}", name=f"kT{i}") for i in range(DC)]
        v_sb = [pers.tile([P, H * VW], fp16, tag=f"v{i}", name=f"v{i}")
                for i in range(SC)]
        otn = [pers.tile([P, S], fp16, tag=f"otn{i}", name=f"otn{i}")
               for i in range(DC)]

        def load_x8(pool, dsrc, prefix, eng):
            tiles = []
            for j in range(NJ):
                t = pool.tile([P, 2, S], fp8e4, tag=f"{prefix}{j}",
                              name=f"{prefix}{j}")
                eng.dma_start(out=t[:], in_=dsrc[j])
                tiles.append(t)
            return tiles

        def load_tiles(pool, dsrc, prefix, bufs=1, eng=None, chunked=False):
            eng = eng or nc.sync
            tiles = []
            for ci in range(DC):
                t = pool.tile([P, D], fp16, tag=f"{prefix}{ci}", bufs=bufs,
                              name=f"{prefix}{ci}")
                src = dsrc[ci] if chunked else dsrc[ci * P:(ci + 1) * P, :]
                eng.dma_start(out=t[:], in_=src)
                tiles.append(t)
            return tiles

        def load_w8_chunk(nm, do, pool, eng=None):
            # host packs chunk do as [128, 2, NJ*128] fp8
            t = pool.tile([P, 2, NJ * P], fp8e4, tag=f"w{nm}c", bufs=2,
                          name=f"w{nm}{do}")
            (eng or nc.sync).dma_start(out=t[:], in_=w_d[nm][do])
            return t

        def qk_proj_mms(wt, xt, ps, sq, j):
            nc.tensor.matmul(
                ps[:, sq * NQ:(sq + 1) * NQ],
                wt[:, :, j * P:(j + 1) * P],
                xt[j][:, :, sq * NQ:(sq + 1) * NQ],
                start=(j == 0), stop=(j == NJ - 1),
                perf_mode=DR,
            )

        def qk_proj_chunk(nm, wt, xt, dst, do, pstag="mm", pbufs=2):
            ps = psmm.tile([P, S], f32, tag=pstag, bufs=pbufs, name=f"ps{nm}{do}")
            for sq in range(2):
                for j in range(NJ):
                    qk_proj_mms(wt, xt, ps, sq, j)
            nc.scalar.activation(
                out=dst[do][:], in_=ps[:], func=AF.Identity,
                bias=bqk_sb[nm][:, do:do + 1],
            )

        def v_proj_chunk(pool, wt, xt_v, sc):
            nc.gpsimd.memset(v_sb[sc][:], 1.0)
            xt = xt_v[sc]
            xct = [xt[:, di * P:(di + 1) * P] for di in range(DC)]
            ps = psmm.tile([P, S], f32, tag="mm", bufs=2, name=f"psv{sc}")
            for dq in range(2):
                for di in range(DC):
                    nc.tensor.matmul(
                        ps[:, dq * NQ:(dq + 1) * NQ],
                        xct[di],
                        wt[di][:, dq * NQ:(dq + 1) * NQ],
                        start=(di == 0),
                        stop=(di == DC - 1),
                    )
            src = ps[:].rearrange("p (a r) -> p a r", r=HD)
            dst = v_sb[sc][:].rearrange("p (a r) -> p a r", r=VW)
            nc.scalar.copy(out=dst[:, :, 0:HD], in_=src[:, :, :])

        def head(h, ap, spair, vwork=None, filler=None):
            hp, hodd = h // 2, h % 2
            pav = psav.tile([P, S], f32, tag="av", bufs=1, name=f"pav{h}")
            pwork = [None] * SC  # p tiles pending PV

            def pv(kc):
                p = pwork[kc]
                for sq in range(2):
                    nc.tensor.matmul(
                        pav[0:VW, sq * NQ:(sq + 1) * NQ],
                        v_sb[kc][:, h * VW:(h + 1) * VW],
                        p[:, sq * NQ:(sq + 1) * NQ],
                        start=(kc == 0), stop=(kc == SC - 1),
                    )

            for kc in range(SC + 1):
                if kc < SC:
                    if vwork is not None:
                        vwork(kc)
                    emsk_t = ap.tile([P, S], fp16, tag="synT", bufs=4,
                                     name=f"em{h}_{kc}")
                    eng = nc.sync if kc % 2 == 0 else nc.gpsimd
                    eng.dma_start(
                        out=emsk_t[:], in_=dram["emsk"][h, kc * P:(kc + 1) * P, :]
                    )
                    ps = psmm.tile([P, S], f32, tag="mm", bufs=2, name="pss")
                    for sq in range(2):
                        nc.tensor.matmul(
                            ps[:, sq * NQ:(sq + 1) * NQ],
                            kT[hp][hodd * HD:(hodd + 1) * HD, kc * P:(kc + 1) * P],
                            qT[hp][hodd * HD:(hodd + 1) * HD, sq * NQ:(sq + 1) * NQ],
                            start=True, stop=True,
                        )
                    p = ap.tile([P, S], fp16, tag="p", bufs=3, name="p")
                    nc.scalar.activation(out=p[:], in_=ps[:], func=AF.Exp,
                                         scale=exp_scale)
                    nc.vector.tensor_tensor(out=p[:], in0=p[:], in1=emsk_t[:],
                                            op=OP.mult)
                    pwork[kc] = p
                if filler is not None:
                    filler()
                if kc > 0:
                    pv(kc - 1)
            # evacuate raw output (ACT) + softmax sums row (DVE)
            nc.scalar.copy(out=otn[hp][hodd * HD:(hodd + 1) * HD, :],
                           in_=pav[0:HD, :])
            nc.vector.tensor_copy(out=spair[32 * hodd:32 * hodd + 1, :],
                                  in_=pav[HD:VW, :])

        def norm(hp, ap, spair):
            # otn[hp] rows 0:64 = head 2hp, 64:128 = head 2hp+1
            # spair rows 1..31 hold 1.0 so the full-tile ops stay finite
            rec = ap.tile([33, S], f32, tag="rec", bufs=1, name=f"rc{hp}")
            nc.vector.reciprocal_approx_fast(out=rec[:], in_=spair[:])
            r16 = ap.tile([33, S], fp16, tag="rec16", bufs=1, name=f"rh{hp}")
            nc.vector.tensor_copy(out=r16[:], in_=rec[:])
            rec16 = [r16[0:1, :], r16[32:33, :]]
            prec = psmm.tile([P, S], f32, tag="pmm", bufs=1, name=f"prc{hp}")
            for r in range(2):
                for sq in range(2):
                    nc.tensor.matmul(
                        prec[r * HD:(r + 1) * HD, sq * NQ:(sq + 1) * NQ],
                        ones_h[32 * r:32 * r + 1, 0:HD],
                        rec16[r][:, sq * NQ:(sq + 1) * NQ],
                        start=True, stop=True,
                    )
            nc.vector.tensor_tensor(out=otn[hp][:], in0=otn[hp][:],
                                    in1=prec[:], op=OP.mult)

        # ================= emission ======================================
        with (
            tc.tile_pool(name="projp", bufs=1) as projp,
            tc.tile_pool(name="attn", bufs=1) as ap,
        ):
            # startup loads spread across DGE queues
            wcq0 = load_w8_chunk("q", 0, projp, eng=nc.sync)
            xq_t = load_x8(projp, dram["xq8"], "xq", nc.sync)
            wck0 = load_w8_chunk("k", 0, projp, eng=nc.sync)
            xk_t = load_x8(projp, dram["xk8"], "xk", nc.scalar)
            wv_t = load_tiles(projp, w_d["v"], "wv", eng=nc.gpsimd)
            xt_v = load_tiles(projp, dram["xv"], "xv", eng=nc.scalar,
                              chunked=True)

            qk_proj_chunk("q", wcq0, xq_t, qT, 0)
            qk_proj_chunk("k", wck0, xk_t, kT, 0)

            # remaining q/k projection chunks drained a few ops per kc slot
            # inside the attention loops (PE stays dense for the HAM clock
            # gate); each chunk's weight DMA is issued one chunk early.
            cw = {}

            def mk_chunk(cid, nm, xt, dst, do):
                def dma():
                    cw[cid] = load_w8_chunk(nm, do, projp)

                def mm(sq, j):
                    key = (cid, "ps")
                    if key not in cw:
                        cw[key] = psmm.tile([P, S], f32, tag="pmm", bufs=1,
                                            name=f"pp{nm}{do}")
                    qk_proj_mms(cw[cid], xt, cw[key], sq, j)

                def evac():
                    nc.scalar.activation(
                        out=dst[do][:], in_=cw[key_ps][:], func=AF.Identity,
                        bias=bqk_sb[nm][:, do:do + 1],
                    )
                key_ps = (cid, "ps")
                body = [lambda sq=sq, j=j: mm(sq, j)
                        for sq in range(2) for j in range(NJ)]
                body.append(evac)
                return dma, body

            chunks = []
            cid = 0
            for do in range(1, DC):
                for nm, xt, dst in (("q", xq_t, qT), ("k", xk_t, kT)):
                    chunks.append((cid, mk_chunk(cid, nm, xt, dst, do)))
                    cid += 1
            # stagger: dma(c0), dma(c1), body(c0), dma(c2), body(c1), ...
            proj_work = []  # list of (chunk_id_done_after_op, fn)
            if chunks:
                proj_work.append((-1, chunks[0][1][0]))
                for i, (ci, (dma, body)) in enumerate(chunks):
                    if i + 1 < len(chunks):
                        proj_work.append((-1, chunks[i + 1][1][0]))
                    for b_idx, fn in enumerate(body):
                        proj_work.append(
                            (ci if b_idx == len(body) - 1 else -1, fn))
            proj_work.reverse()  # pop() from the end

            done_cid = [-1]

            def pop_one():
                ci, fn = proj_work.pop()
                fn()
                if ci >= 0:
                    done_cid[0] = ci

            def drain(n):
                def f():
                    for _ in range(n):
                        if proj_work:
                            pop_one()
                return f

            def force_until(cid_needed):
                while proj_work and done_cid[0] < cid_needed:
                    pop_one()

            wt_o = None
            spairs = {}

            for hp in range(DC):
                if hp == 1:
                    wt_o = load_tiles(projp, w_d["o"], "wo", chunked=True)
                spair = ap.tile([33, S], f32, tag="spair", bufs=2,
                                name=f"sp{hp}")
                spairs[hp] = spair
                nc.gpsimd.memset(spair[:], 1.0)
                if hp == 0:
                    head(0, ap, spair,
                         vwork=lambda kc: v_proj_chunk(projp, wv_t, xt_v, kc),
                         filler=drain(2))
                else:
                    # chunks q/k(hp) must be fully emitted before this pair
                    force_until(2 * hp - 1)
                    head(2 * hp, ap, spair, filler=drain(2))
                if hp > 0:
                    # deferred: previous pair's normalization hides behind
                    # this pair's attention stream
                    norm(hp - 1, ap, spairs.pop(hp - 1))
                head(2 * hp + 1, ap, spair, filler=drain(2))
            while proj_work:
                pop_one()
            norm(DC - 1, ap, spairs.pop(DC - 1))

            # ============= output projection (transposed) ================
            # out^T[do*P+p, s] = sum_d Wo[d, do*P+p] * otn[d, s] + boeff
            for do in range(DC):
                ps = psmm.tile([P, S], f32, tag="mm", bufs=2, name=f"pso{do}")
                for sq in range(2):
                    for di in range(DC):
                        nc.tensor.matmul(
                            ps[:, sq * NQ:(sq + 1) * NQ],
                            wt_o[do][:, di * P:(di + 1) * P],
                            otn[di][:, sq * NQ:(sq + 1) * NQ],
                            start=(di == 0), stop=(di == DC - 1),
                        )
                osb = ap.tile([P, S], fp16, tag="osb", bufs=2, name="osb")
                nc.scalar.activation(
                    out=osb[:], in_=ps[:], func=AF.Identity,
                    bias=bo_sb[:, do:do + 1],
                )
                nc.sync.dma_start(out=out_d[do * P:(do + 1) * P, :], in_=osb[:])


def _build(exp_scale):
    nc = bacc.Bacc("TRN2", debug=False)
    dram = {
        "xq8": nc.declare_dram_parameter("xq8", [NJ, P, 2, S], fp8e4, isOutput=False),
        "xk8": nc.declare_dram_parameter("xk8", [NJ, P, 2, S], fp8e4, isOutput=False),
        "xv": nc.declare_dram_parameter("xv", [SC, P, D], fp16, isOutput=False),
        "wq": nc.declare_dram_parameter("wq", [DC, P, 2, NJ * P], fp8e4, isOutput=False),
        "wk": nc.declare_dram_parameter("wk", [DC, P, 2, NJ * P], fp8e4, isOutput=False),
        "wv": nc.declare_dram_parameter("wv", [D, D], fp16, isOutput=False),
        "wo": nc.declare_dram_parameter("wo", [DC, P, D], fp16, isOutput=False),
        "bq": nc.declare_dram_parameter("bq", [D], f32, isOutput=False),
        "bk": nc.declare_dram_parameter("bk", [D], f32, isOutput=False),
        "boeff": nc.declare_dram_parameter("boeff", [D], f32, isOutput=False),
        "emsk": nc.declare_dram_parameter("emsk", [H, S, S], fp16, isOutput=False),
        "out": nc.declare_dram_parameter("out", [D, S], fp16, isOutput=True),
    }
    with tile.TileContext(nc) as tc:
        _emit(nc, tc, dram, exp_scale)
    nc.compile()
    return nc


def _prep(inputs):
    import ml_dtypes

    q = np.asarray(inputs["query"], np.float32)
    k = np.asarray(inputs["key"], np.float32)
    v = np.asarray(inputs["value"], np.float32)
    msk = np.asarray(inputs["mask"], np.int32)
    ws = {nm: np.asarray(inputs["W" + nm], np.float32) for nm in "qkvo"}
    bs = {nm: np.asarray(inputs["b" + nm], np.float32) for nm in "qkvo"}
    alpha = 1.0 / (1.0 + math.exp(-float(np.asarray(inputs["alpha_param"]).ravel()[0])))
    c1 = alpha / math.sqrt(HD)
    c2 = 1.0 - alpha
    exp_scale = c1 / (SCL * SCL)

    # esynT[h][k, q] = exp(c2 * syn[h][q, k]) in fp16
    esynT = np.exp(
        c2 * np.asarray(inputs["syn_scores"], np.float32)[:, :S, :S].transpose(0, 2, 1)
    ).astype(np.float16)
    boeff = (bs["v"].astype(np.float64) @ ws["o"].astype(np.float64)
             + bs["o"]).astype(np.float32)

    def chunk_pack(w):
        # [do, p, di*P + c] = w[di*P + p, do*P + c]
        w4 = w.reshape(DC, P, DC, P)          # [di, p, do, c]
        return np.ascontiguousarray(
            w4.transpose(2, 1, 0, 3).reshape(DC, P, D))

    def w8_pack(w):
        # [do, p, i, j*P + c] = w[(2j+i)*P + p, do*P + c]
        w6 = w.reshape(NJ, 2, P, DC, P)       # [j, i, p, do, c]
        return np.ascontiguousarray(
            w6.transpose(3, 2, 1, 0, 4).reshape(DC, P, 2, NJ * P)
        ).astype(ml_dtypes.float8_e4m3)

    def x8_pack(xT):
        # xT: [D, S] -> [j, p, i, s] with d = (2j+i)*P + p
        x4 = xT.reshape(NJ, 2, P, S)
        return np.ascontiguousarray(
            x4.transpose(0, 2, 1, 3)).astype(ml_dtypes.float8_e4m3)

    common = {
        "wq": w8_pack(SCL * ws["q"]),
        "wk": w8_pack(SCL * ws["k"]),
        "wv": ws["v"].astype(np.float16),
        "wo": chunk_pack(ws["o"].astype(np.float16)),
        "bq": SCL * bs["q"],
        "bk": SCL * bs["k"],
        "boeff": boeff,
    }
    in_maps = []
    for b in range(B):
        m = dict(common)
        m["xq8"] = x8_pack(q[b].T)
        m["xk8"] = x8_pack(k[b].T)
        m["xv"] = chunk_pack(v[b].T.astype(np.float16))
        # emsk[h][k, q] = esynT[h][k, q] * mask[b][q, k]
        mTb = np.ascontiguousarray(msk[b].T).astype(np.float16)
        m["emsk"] = esynT * mTb[None, :, :]
        in_maps.append(m)
    return in_maps, exp_scale


def kernel(**inputs):
    global LAST_RESULTS
    in_maps, exp_scale = _prep(inputs)
    key = ("nc", exp_scale)
    if key not in _CACHE:
        _CACHE[key] = _build(exp_scale)
    nc = _CACHE[key]

    kwargs = {}
    if TRACE:
        kwargs["trace"] = True
        if TRACE_TMPDIR:
            kwargs["tmpdir"] = TRACE_TMPDIR
    res = run_bass_kernel_spmd(nc, in_maps, core_ids=list(range(N_CORES)), **kwargs)
    LAST_RESULTS = res
    return np.stack(
        [res.results[b]["out"].astype(np.float32).T for b in range(B)], axis=0
    )
